# revision 12
# baseline (speedup 1.0000x reference)
"""Trainium2 Bass kernel for nn_MultiHeadAttention_50861002719805. v9.

Full inputs in, full output out. 8 cores = 4 batches x 2 head-groups; each
core computes attention for its batch + 8 heads, pairs exchange normalized
per-head outputs via AllGather, each core projects all 16 heads into its 512
output columns.

Per-core pipeline (all phases expressed as micro-op streams so the in-order
engine queues always have fill work):
- x resident in SBUF: bf16 for the V/Q matmuls, fp8-packed (d-pairs) for the
  K projection which runs fp8 DoubleRow (2x).  Scores K^T Q in bf16 with two
  heads row-packed; exp on the scalar engine carries the 1/32 scale; masked
  entries become exp(1e-9)=1.0 via copy_predicated + rank-1 suffix/block
  corrections (coarse diagonal split: the all-ones half of the second
  diagonal cp-group is skipped and added as one rank-1).
- rep r+1's prelude (input DMA, V phase, suffix sums, QK of pair 0) is
  interleaved into rep r's last attention pair; rep r's output projection is
  interleaved into rep r+1's first attention pair, so at steady state the
  scalar engine (exp) never starves.
"""
import numpy as np
import ml_dtypes

import concourse.bacc as bacc
import concourse.mybir as mybir
import concourse.tile as tile
from concourse.bass_utils import run_bass_kernel_spmd

F32 = mybir.dt.float32
BF16 = mybir.dt.bfloat16
U8 = mybir.dt.uint8
F8 = mybir.dt.float8e4
DR = mybir.MatmulPerfMode.DoubleRow

B, T, D = 4, 2048, 1024
H, HS = 16, 64
HL = 8
TCH, SCH = 512, 128
NTC, NSC = T // TCH, T // SCH   # 4, 16
NDC = D // 128                  # 8
NDC2 = 4                        # fp8 DoubleRow d-pair chunks
NP = 4
HWID = 4 * (HS + 1)             # 260
MULT = mybir.AluOpType.mult
BYPASS = mybir.AluOpType.bypass
GROUPS = [[0, 1], [2, 3], [4, 5], [6, 7]]
EXP = mybir.ActivationFunctionType.Exp


def build(reps=1, collective=True):
    nc = bacc.Bacc("TRN2", target_bir_lowering=False, debug=False, num_devices=8)

    xT = nc.declare_dram_parameter("xT", [D, T], BF16, isOutput=False)
    xT8 = nc.declare_dram_parameter("xT8", [128, NDC2, 2, T], F8, isOutput=False)
    wq = nc.declare_dram_parameter("wq", [D, HL * HS], BF16, isOutput=False)
    wk = nc.declare_dram_parameter("wk", [128, NDC2, 2, HL * HS], F8,
                                   isOutput=False)
    wv = nc.declare_dram_parameter("wv", [D, HL * HS], BF16, isOutput=False)
    wo = nc.declare_dram_parameter("wo", [D, TCH], BF16, isOutput=False)
    mask = nc.declare_dram_parameter("mask", [4, SCH, TCH], U8, isOutput=False)
    out = nc.declare_dram_parameter("out", [T, TCH], BF16, isOutput=True)

    with tile.TileContext(nc) as tc, (
        tc.tile_pool(name="const", bufs=2)) as cpool, (
        tc.tile_pool(name="wpool", bufs=1)) as wpool, (
        tc.tile_pool(name="vstp", bufs=2)) as vstp, (
        tc.tile_pool(name="osbp", bufs=1)) as osbp, (
        tc.tile_pool(name="small", bufs=2)) as sp, (
        tc.tile_pool(name="dram", bufs=1, space="DRAM")) as dp, (
        tc.tile_pool(name="spool", bufs=2, space="PSUM")) as spool, (
        tc.tile_pool(name="qpool", bufs=1, space="PSUM")) as qpool, (
        tc.tile_pool(name="opool", bufs=2, space="PSUM")) as opool, (
        tc.tile_pool(name="qkt", bufs=2)) as qkt, (
        tc.tile_pool(name="ep", bufs=2)) as ep:

        def flush(ops, n=None):
            k = len(ops) if n is None else min(n, len(ops))
            for _ in range(k):
                ops.pop(0)()

        def qk_ops(res, rep, j):
            """QK projection micro-ops for pair j (Q bf16, K fp8 DoubleRow)."""
            QT = qkt.tile([128, NTC, TCH], BF16, tag="qt", name=f"QT{rep}_{j}")
            KT = qkt.tile([128, NTC, TCH], BF16, tag="kt", name=f"KT{rep}_{j}")
            x_sb, x8_sb = res["x_sb"], res["x8_sb"]
            wq_sb, wk_sb = res["wq_sb"], res["wk_sb"]
            ops = []
            state = {}
            for tcb in range(NTC):
                def alloc(tcb=tcb):
                    state[tcb] = (
                        qpool.tile([128, TCH], F32, tag="pq",
                                   name=f"pq{rep}_{j}_{tcb}"),
                        qpool.tile([128, TCH], F32, tag="pk",
                                   name=f"pk{rep}_{j}_{tcb}"))
                ops.append(alloc)
                for dc in range(NDC):
                    def mm(tcb=tcb, dc=dc):
                        pq, pk = state[tcb]
                        nc.tensor.matmul(
                            pq[:], wq_sb[:, dc, j * 128:(j + 1) * 128],
                            x_sb[:, dc, tcb * TCH:(tcb + 1) * TCH],
                            start=(dc == 0), stop=(dc == NDC - 1))
                        if dc < NDC2:
                            nc.tensor.matmul(
                                pk[:],
                                wk_sb[:, dc, :, j * 128:(j + 1) * 128],
                                x8_sb[:, dc, :, tcb * TCH:(tcb + 1) * TCH],
                                start=(dc == 0), stop=(dc == NDC2 - 1),
                                perf_mode=DR)
                    ops.append(mm)
                def cp_out(tcb=tcb):
                    pq, pk = state[tcb]
                    nc.vector.tensor_copy(QT[:, tcb, :], pq[:])
                    nc.vector.tensor_copy(KT[:, tcb, :], pk[:])
                ops.append(cp_out)
            return QT, KT, ops

        def make_prelude(rep):
            """Allocate rep-scoped tiles; micro-ops for input DMA + V phase +
            suffix/pair sums + pair-0 QK."""
            res = {}
            res["o_my"] = [dp.tile([128, T], BF16, name=f"omy{rep}_{j}")
                           for j in range(NP)]
            res["o_all"] = [dp.tile([2, 128, T], BF16, name=f"oall{rep}_{j}")
                            for j in range(NP)]
            x_sb = res["x_sb"] = wpool.tile([128, NDC, T], BF16,
                                            name=f"x_sb{rep}", tag="x")
            x8_sb = res["x8_sb"] = wpool.tile([128, NDC2, 2, T], F8,
                                              name=f"x8_sb{rep}", tag="x8")
            wq_sb = res["wq_sb"] = wpool.tile([128, NDC, HL * HS], BF16,
                                              name=f"wq_sb{rep}", tag="wq")
            wk_sb = res["wk_sb"] = wpool.tile([128, NDC2, 2, HL * HS], F8,
                                              name=f"wk_sb{rep}", tag="wk")
            wv_sb = res["wv_sb"] = wpool.tile([128, NDC, HL * HS], BF16,
                                              name=f"wv_sb{rep}", tag="wv")
            wo_sb = res["wo_sb"] = wpool.tile([128, NDC, TCH], BF16,
                                              name=f"wo_sb{rep}", tag="wo")
            mask_sb = res["mask_sb"] = cpool.tile([SCH, 4, TCH], U8,
                                                  name=f"mask{rep}",
                                                  tag="mask", bufs=1)
            ones_col = res["ones_col"] = cpool.tile([128, 1], BF16,
                                                    name=f"onec{rep}", tag="onec")
            ones_t = res["ones_t"] = cpool.tile([128, TCH], BF16,
                                                name=f"onet{rep}", tag="onet")
            V_st = res["V_st"] = vstp.tile([SCH, NSC, HL, HS + 1], BF16,
                                           name=f"V_st{rep}", tag="V")
            res["O_sb"] = osbp.tile([128, 2, NP, T], BF16,
                                    name=f"O_sb{rep}", tag="O")
            vsuf = res["vsuf"] = cpool.tile([1, 3, 2, HWID], BF16,
                                            name=f"vsuf{rep}", tag="vsuf")
            vpr = res["vpr"] = cpool.tile([1, NTC, 2, HWID], BF16,
                                          name=f"vpr{rep}", tag="vpr")

            ops = []
            for dc in range(NDC):
                def dma_in(dc=dc):
                    nc.sync.dma_start(wv_sb[:, dc, :],
                                      wv[dc * 128:(dc + 1) * 128, :])
                    nc.sync.dma_start(x_sb[:, dc, :],
                                      xT[dc * 128:(dc + 1) * 128, :])
                ops.append(dma_in)

            def dma_misc():
                nc.vector.memset(ones_col[:], 1.0)
                for dc in range(NDC):
                    nc.sync.dma_start(wq_sb[:, dc, :],
                                      wq[dc * 128:(dc + 1) * 128, :])
                for dc in range(NDC2):
                    nc.sync.dma_start(x8_sb[:, dc, :, :], xT8[:, dc, :, :])
                    nc.sync.dma_start(wk_sb[:, dc, :, :], wk[:, dc, :, :])
            ops.append(dma_misc)

            # V phase: dc-outer over groups of 4 s-chunks
            vstate = {}
            for grp in range(4):
                def v_alloc(grp=grp):
                    vstate[grp] = [
                        spool.tile([SCH, 2, HL, HS], F32, tag="ps",
                                   name=f"pv{rep}_{grp}_{i}") for i in range(2)]
                ops.append(v_alloc)
                for dc in range(NDC):
                    def v_mm(grp=grp, dc=dc):
                        pv = vstate[grp]
                        for i in range(2):
                            for u in range(2):
                                sc = 4 * grp + 2 * i + u
                                nc.tensor.matmul(
                                    pv[i][:, u, :, :],
                                    x_sb[:, dc, sc * 128:(sc + 1) * 128],
                                    wv_sb[:, dc, :],
                                    start=(dc == 0), stop=(dc == NDC - 1),
                                    skip_group_check=True)
                    ops.append(v_mm)
                if grp == 0:
                    def v_memset():
                        # first write of the rep into V_st (WARs on the
                        # previous rep's last attention reads)
                        nc.vector.memset(V_st[:, :, :, 0:1], 1.0)
                    ops.append(v_memset)
                def v_copy(grp=grp):
                    pv = vstate[grp]
                    for i in range(2):
                        for u in range(2):
                            nc.vector.tensor_copy(
                                V_st[:, 4 * grp + 2 * i + u, :, 1:HS + 1],
                                pv[i][:, u, :, :])
                ops.append(v_copy)

            # suffix sums over s-chunk tails (incl. masked counts in col 0)
            for tcb in range(3):
                for half in range(2):
                    def sfx(tcb=tcb, half=half):
                        psf = opool.tile([1, HWID], F32, tag="po",
                                         name=f"psf{rep}_{tcb}_{half}")
                        lo = 4 * (tcb + 1)
                        for c in range(lo, NSC):
                            nc.tensor.matmul(
                                psf[:], ones_col[:],
                                V_st[:, c, half * 4:(half + 1) * 4, :],
                                start=(c == lo), stop=(c == NSC - 1))
                        nc.vector.tensor_copy(vsuf[0:1, tcb, half, :], psf[:])
                    ops.append(sfx)
            # pair sums of chunks (4t+2, 4t+3): their t<256 region is all-ones
            for tcb in range(NTC):
                for half in range(2):
                    def prs(tcb=tcb, half=half):
                        ppr = opool.tile([1, HWID], F32, tag="po",
                                         name=f"ppr{rep}_{tcb}_{half}")
                        for ci, c in enumerate((4 * tcb + 2, 4 * tcb + 3)):
                            nc.tensor.matmul(
                                ppr[:], ones_col[:],
                                V_st[:, c, half * 4:(half + 1) * 4, :],
                                start=(ci == 0), stop=(ci == 1))
                        nc.vector.tensor_copy(vpr[0:1, tcb, half, :], ppr[:])
                    ops.append(prs)

            def dma_late():
                nc.vector.memset(ones_t[:], 1.0)
                for k in range(4):
                    nc.sync.dma_start(mask_sb[:, k, :], mask[k, :, :])
                for dc in range(NDC):
                    nc.sync.dma_start(wo_sb[:, dc, :],
                                      wo[dc * 128:(dc + 1) * 128, :])
            ops.append(dma_late)

            QT, KT, qops = qk_ops(res, rep, 0)
            res["qk0"] = (QT, KT)
            ops.extend(qops)
            return res, ops

        def norm_rest(res, rep, j, tcb, stg):
            # deferred tail of the normalize: off the po critical path
            def run():
                rp0, rbc, og = [], [], []
                for e in range(2):
                    rp0.append(sp.tile([1, TCH], F32, tag="rp0",
                                       name=f"rp0_{rep}_{j}_{tcb}_{e}"))
                    nc.vector.reciprocal(rp0[e][:], stg[e][0:1, :])
                for e in range(2):
                    rbc.append(sp.tile([HS + 1, TCH], F32, tag="rbc",
                                       name=f"rbc_{rep}_{j}_{tcb}_{e}"))
                    nc.gpsimd.partition_broadcast(
                        rbc[e][:], rp0[e][:], channels=HS + 1)
                for e in range(2):
                    og.append(sp.tile([HS + 1, TCH], BF16, tag="og",
                                      name=f"og_{rep}_{j}_{tcb}_{e}"))
                    nc.vector.tensor_tensor(
                        og[e][:], stg[e][:], rbc[e][:], MULT)
                for e in range(2):
                    nc.sync.dma_start(
                        res["o_my"][j][64 * e:64 * e + 64,
                                       tcb * TCH:(tcb + 1) * TCH],
                        og[e][1:HS + 1, :])
                if tcb == NTC - 1:
                    if collective:
                        nc.gpsimd.collective_compute(
                            "AllGather", BYPASS,
                            replica_groups=GROUPS,
                            ins=[res["o_my"][j][:]],
                            outs=[res["o_all"][j][:]],
                        )
                    for g in range(2):
                        src = (res["o_all"][j][g, :, :] if collective
                               else res["o_my"][j][:])
                        nc.sync.dma_start(res["O_sb"][:, g, j, :], src)
            return run

        def proj_ops(res, rep):
            """Output projection micro-ops (one per 128-row t-tile)."""
            O_sb, wo_sb = res["O_sb"], res["wo_sb"]
            jj_order = [g * 4 + jp for jp in range(NP) for g in range(2)]
            ops = []
            for tt in range(T // 128):
                def run(tt=tt):
                    pp = qpool.tile([128, TCH], F32,
                                    tag=("pq" if tt % 2 == 0 else "pk"),
                                    name=f"pp{rep}_{tt}")
                    for i, jj in enumerate(jj_order):
                        g, jp = jj // 4, jj % 4
                        nc.tensor.matmul(
                            pp[:],
                            O_sb[:, g, jp, tt * 128:(tt + 1) * 128],
                            wo_sb[:, jj, :],
                            start=(i == 0), stop=(i == NDC - 1))
                    ob = sp.tile([128, TCH], BF16, tag="ob",
                                 name=f"ob{rep}_{tt}")
                    nc.vector.tensor_copy(ob[:], pp[:])
                    nc.sync.dma_start(out[tt * 128:(tt + 1) * 128, :], ob[:])
                ops.append(run)
            return ops

        # ================= main pipeline =================
        res, pre = make_prelude(0)
        flush(pre)
        carry_proj = []

        pending_norm = []
        for rep in range(reps):
            for j in range(NP):
                if j + 1 < NP:
                    QTn, KTn, nxt = qk_ops(res, rep, j + 1)
                elif rep + 1 < reps:
                    res_next, nxt = make_prelude(rep + 1)
                    QTn = KTn = None
                else:
                    res_next, nxt, QTn, KTn = None, [], None, None
                QT, KT = res["qk0"] if j == 0 else (QT, KT)
                mask_sb = res["mask_sb"]
                ones_t = res["ones_t"]
                V_st = res["V_st"]
                vsuf, vpr = res["vsuf"], res["vpr"]
                # ---- attention for heads (2j, 2j+1) ----
                for tcb in range(NTC):
                    nv = 4 * (tcb + 1)
                    E = [ep.tile([SCH, 8, TCH], BF16, tag="E",
                                 name=f"E{rep}_{j}_{tcb}_{ee}")
                         for ee in range(2)]
                    po = [opool.tile([HS + 1, TCH], F32, tag="po",
                                     name=f"po{rep}_{j}_{tcb}_{ee}")
                          for ee in range(2)]
                    for cp in range(nv // 2):
                        ps = [None, None]
                        for e in range(2):
                            ps[e] = spool.tile(
                                [SCH, 2, TCH], F32, tag="ps",
                                name=f"ps{rep}_{j}_{tcb}_{cp}_{e}")
                        if cp == 1 and pending_norm:
                            pending_norm.pop(0)()
                        # coarse diagonal split: the second diagonal cp-group
                        # computes only t >= 256; its t < 256 all-ones region
                        # is added as one rank-1 below
                        g = cp - 2 * tcb
                        t0 = 256 if g == 1 else 0
                        for u in range(2):
                            for e in range(2):
                                c = 2 * cp + u
                                nc.tensor.matmul(
                                    ps[e][:, u, t0:TCH],
                                    KT[64 * e:64 * e + 64, c // 4,
                                       (c % 4) * SCH:(c % 4 + 1) * SCH],
                                    QT[64 * e:64 * e + 64, tcb, t0:TCH],
                                    start=True, stop=True)
                        flush(nxt, 2)
                        if not nxt and carry_proj:
                            # only once the QK/prelude stream for this window
                            # has fully drained: proj shares the qpool tags,
                            # and interleaving the allocations deadlocks the
                            # in-order queues
                            flush(carry_proj, 2)
                        for e in range(2):
                            cm = (2 * cp) % 8
                            nc.scalar.activation(
                                E[e][:, cm:cm + 2, t0:TCH],
                                ps[e][:, :, t0:TCH], EXP, scale=1.0 / 32.0)
                            if g >= 0:
                                k0, k1 = 2 * g, 2 * g + 1
                                nc.vector.copy_predicated(
                                    E[e][:, cm, t0:t0 + 128],
                                    mask_sb[:, k0, t0:t0 + 128],
                                    ones_t[:, 0:128])
                                nc.vector.copy_predicated(
                                    E[e][:, cm + 1, t0:t0 + 256],
                                    mask_sb[:, k1, t0:t0 + 256],
                                    ones_t[:, 0:256])
                        for e in range(2):
                            h = 2 * j + e
                            for u in range(2):
                                c = 2 * cp + u
                                nc.tensor.matmul(
                                    po[e][:, t0:TCH],
                                    V_st[:, c, h, :],
                                    E[e][:, c % 8, t0:TCH],
                                    start=(c == 0), stop=False,
                                    skip_group_check=True)
                            if g == 1:
                                # all-ones region of chunks (4t+2, 4t+3)
                                nc.tensor.matmul(
                                    po[e][:, 0:256],
                                    vpr[0:1, tcb, j // 2,
                                        (h % 4) * (HS + 1):
                                        (h % 4 + 1) * (HS + 1)],
                                    ones_t[0:1, 0:256],
                                    start=False, stop=(tcb == 3),
                                    skip_group_check=True)
                        flush(nxt, 2)
                    if tcb < 3:
                        for e in range(2):
                            h = 2 * j + e
                            nc.tensor.matmul(
                                po[e][:],
                                vsuf[0:1, tcb, j // 2,
                                     (h % 4) * (HS + 1):
                                     (h % 4 + 1) * (HS + 1)],
                                ones_t[0:1, :],
                                start=False, stop=True,
                                skip_group_check=True)
                    # normalize: fast psum->sbuf staging frees the po bank;
                    # the recip/broadcast/mult tail is deferred
                    stg = []
                    for e in range(2):
                        stg.append(sp.tile([HS + 1, TCH], F32, tag="stg",
                                           name=f"stg{rep}_{j}_{tcb}_{e}"))
                        nc.vector.tensor_copy(stg[e][:], po[e][:])
                    pending_norm.append(norm_rest(res, rep, j, tcb, stg))
                flush(nxt)
                if j + 1 < NP:
                    QT, KT = QTn, KTn
            # this rep's projection: interleave into the next rep's first
            # attention pair (or flush now on the last rep)
            carry_proj.extend(proj_ops(res, rep))
            if rep + 1 < reps:
                res = res_next
            else:
                while pending_norm:
                    pending_norm.pop(0)()
                flush(carry_proj)

    nc.compile()
    return nc


def make_mask():
    p = np.arange(SCH)[:, None]
    f = np.arange(TCH)[None, :]
    return np.stack([(p + 128 * k > f) for k in range(4)]).astype(np.uint8)


def make_in_maps(x, W_qkv, W_out):
    x = np.asarray(x, dtype=np.float32)
    W_qkv = np.asarray(W_qkv, dtype=np.float32)
    W_out = np.asarray(W_out, dtype=np.float32)
    mask = make_mask()
    in_maps = []
    for c in range(8):
        b, hg = c // 2, c % 2
        heads = slice(hg * HL, (hg + 1) * HL)
        wq_h = W_qkv[heads, :, 0:HS].transpose(1, 0, 2).reshape(D, HL * HS)
        wk_h = W_qkv[heads, :, HS:2 * HS].transpose(1, 0, 2).reshape(D, HL * HS)
        wv_h = W_qkv[heads, :, 2 * HS:3 * HS].transpose(1, 0, 2).reshape(D, HL * HS)
        bf = ml_dtypes.bfloat16
        f8 = ml_dtypes.float8_e4m3

        def pack(a):
            # [D, n] -> [128, 4, 2, n] with d = dcp*256 + 2*p + i
            return np.ascontiguousarray(
                a.reshape(4, 128, 2, a.shape[1]).transpose(1, 0, 2, 3)
            ).astype(f8)

        in_maps.append({
            "xT": np.ascontiguousarray(x[b].T).astype(bf),
            "xT8": pack(x[b].T),
            "wq": np.ascontiguousarray(wq_h).astype(bf),
            "wk": pack(wk_h),
            "wv": np.ascontiguousarray(wv_h).astype(bf),
            "wo": np.ascontiguousarray(
                W_out[:, hg * TCH:(hg + 1) * TCH]).astype(bf),
            "mask": mask,
        })
    return in_maps


_NC_CACHE = {}


def get_nc():
    if "nc" not in _NC_CACHE:
        _NC_CACHE["nc"] = build()
    return _NC_CACHE["nc"]


def kernel(x, W_qkv, W_out):
    nc = get_nc()
    in_maps = make_in_maps(x, W_qkv, W_out)
    res = run_bass_kernel_spmd(nc, in_maps, list(range(8)))
    out = np.empty((B, T, D), dtype=np.float32)
    for b in range(B):
        out[b, :, 0:TCH] = np.asarray(res.results[2 * b]["out"],
                                      dtype=np.float32)
        out[b, :, TCH:D] = np.asarray(res.results[2 * b + 1]["out"],
                                      dtype=np.float32)
    return out


# revision 13
# speedup vs baseline: 1.0291x; 1.0291x over previous
"""Trainium2 Bass kernel for nn_MultiHeadAttention_50861002719805. v9.

Full inputs in, full output out. 8 cores = 4 batches x 2 head-groups; each
core computes attention for its batch + 8 heads, pairs exchange normalized
per-head outputs via AllGather, each core projects all 16 heads into its 512
output columns.

Per-core pipeline (all phases expressed as micro-op streams so the in-order
engine queues always have fill work):
- x resident in SBUF: bf16 for the V/Q matmuls, fp8-packed (d-pairs) for the
  K projection which runs fp8 DoubleRow (2x).  Scores K^T Q in bf16 with two
  heads row-packed; exp on the scalar engine carries the 1/32 scale; masked
  entries become exp(1e-9)=1.0 via copy_predicated + rank-1 suffix/block
  corrections (coarse diagonal split: the all-ones half of the second
  diagonal cp-group is skipped and added as one rank-1).
- rep r+1's prelude (input DMA, V phase, suffix sums, QK of pair 0) is
  interleaved into rep r's last attention pair; rep r's output projection is
  interleaved into rep r+1's first attention pair, so at steady state the
  scalar engine (exp) never starves.
"""
import numpy as np
import ml_dtypes

import concourse.bacc as bacc
import concourse.mybir as mybir
import concourse.tile as tile
from concourse.bass_utils import run_bass_kernel_spmd

F32 = mybir.dt.float32
BF16 = mybir.dt.bfloat16
U8 = mybir.dt.uint8
F8 = mybir.dt.float8e4
DR = mybir.MatmulPerfMode.DoubleRow

B, T, D = 4, 2048, 1024
H, HS = 16, 64
HL = 8
TCH, SCH = 512, 128
NTC, NSC = T // TCH, T // SCH   # 4, 16
NDC = D // 128                  # 8
NDC2 = 4                        # fp8 DoubleRow d-pair chunks
NP = 4
HWID = 4 * (HS + 1)             # 260
MULT = mybir.AluOpType.mult
BYPASS = mybir.AluOpType.bypass
GROUPS = [[0, 1], [2, 3], [4, 5], [6, 7]]
EXP = mybir.ActivationFunctionType.Exp


def build(reps=1, collective=True):
    nc = bacc.Bacc("TRN2", target_bir_lowering=False, debug=False, num_devices=8)

    xT = nc.declare_dram_parameter("xT", [D, T], BF16, isOutput=False)
    xT8 = nc.declare_dram_parameter("xT8", [128, NDC2, 2, T], F8, isOutput=False)
    wq = nc.declare_dram_parameter("wq", [D, HL * HS], BF16, isOutput=False)
    wk = nc.declare_dram_parameter("wk", [128, NDC2, 2, HL * HS], F8,
                                   isOutput=False)
    wv = nc.declare_dram_parameter("wv", [D, HL * HS], BF16, isOutput=False)
    wo = nc.declare_dram_parameter("wo", [D, TCH], BF16, isOutput=False)
    mask = nc.declare_dram_parameter("mask", [4, SCH, TCH], U8, isOutput=False)
    out = nc.declare_dram_parameter("out", [T, TCH], BF16, isOutput=True)

    with tile.TileContext(nc) as tc, (
        tc.tile_pool(name="const", bufs=2)) as cpool, (
        tc.tile_pool(name="wpool", bufs=1)) as wpool, (
        tc.tile_pool(name="vstp", bufs=2)) as vstp, (
        tc.tile_pool(name="osbp", bufs=1)) as osbp, (
        tc.tile_pool(name="small", bufs=2)) as sp, (
        tc.tile_pool(name="dram", bufs=1, space="DRAM")) as dp, (
        tc.tile_pool(name="spool", bufs=2, space="PSUM")) as spool, (
        tc.tile_pool(name="qpool", bufs=1, space="PSUM")) as qpool, (
        tc.tile_pool(name="opool", bufs=2, space="PSUM")) as opool, (
        tc.tile_pool(name="qkt", bufs=2)) as qkt, (
        tc.tile_pool(name="ep", bufs=2)) as ep:

        def flush(ops, n=None):
            k = len(ops) if n is None else min(n, len(ops))
            for _ in range(k):
                ops.pop(0)()

        def qk_ops(res, rep, j):
            """QK projection micro-ops for pair j (Q bf16, K fp8 DoubleRow)."""
            QT = qkt.tile([128, NTC, TCH], BF16, tag="qt", name=f"QT{rep}_{j}")
            KT = qkt.tile([128, NTC, TCH], BF16, tag="kt", name=f"KT{rep}_{j}")
            x_sb, x8_sb = res["x_sb"], res["x8_sb"]
            wq_sb, wk_sb = res["wq_sb"], res["wk_sb"]
            ops = []
            state = {}
            for tcb in range(NTC):
                def alloc(tcb=tcb):
                    state[tcb] = (
                        qpool.tile([128, TCH], F32, tag="pq",
                                   name=f"pq{rep}_{j}_{tcb}"),
                        qpool.tile([128, TCH], F32, tag="pk",
                                   name=f"pk{rep}_{j}_{tcb}"))
                ops.append(alloc)
                for dc in range(NDC):
                    def mm(tcb=tcb, dc=dc):
                        pq, pk = state[tcb]
                        nc.tensor.matmul(
                            pq[:], wq_sb[:, dc, j * 128:(j + 1) * 128],
                            x_sb[:, dc, tcb * TCH:(tcb + 1) * TCH],
                            start=(dc == 0), stop=(dc == NDC - 1))
                        if dc < NDC2:
                            nc.tensor.matmul(
                                pk[:],
                                wk_sb[:, dc, :, j * 128:(j + 1) * 128],
                                x8_sb[:, dc, :, tcb * TCH:(tcb + 1) * TCH],
                                start=(dc == 0), stop=(dc == NDC2 - 1),
                                perf_mode=DR)
                    ops.append(mm)
                def cp_out(tcb=tcb):
                    pq, pk = state[tcb]
                    nc.vector.tensor_copy(QT[:, tcb, :], pq[:])
                    nc.vector.tensor_copy(KT[:, tcb, :], pk[:])
                ops.append(cp_out)
            return QT, KT, ops

        def make_prelude(rep):
            """Allocate rep-scoped tiles; micro-ops for input DMA + V phase +
            suffix/pair sums + pair-0 QK."""
            res = {}
            res["o_my"] = [dp.tile([128, T], BF16, name=f"omy{rep}_{j}")
                           for j in range(NP)]
            res["o_all"] = [dp.tile([2, 128, T], BF16, name=f"oall{rep}_{j}")
                            for j in range(NP)]
            x_sb = res["x_sb"] = wpool.tile([128, NDC, T], BF16,
                                            name=f"x_sb{rep}", tag="x")
            x8_sb = res["x8_sb"] = wpool.tile([128, NDC2, 2, T], F8,
                                              name=f"x8_sb{rep}", tag="x8")
            wq_sb = res["wq_sb"] = wpool.tile([128, NDC, HL * HS], BF16,
                                              name=f"wq_sb{rep}", tag="wq")
            wk_sb = res["wk_sb"] = wpool.tile([128, NDC2, 2, HL * HS], F8,
                                              name=f"wk_sb{rep}", tag="wk")
            wv_sb = res["wv_sb"] = wpool.tile([128, NDC, HL * HS], BF16,
                                              name=f"wv_sb{rep}", tag="wv")
            wo_sb = res["wo_sb"] = wpool.tile([128, NDC, TCH], BF16,
                                              name=f"wo_sb{rep}", tag="wo")
            mask_sb = res["mask_sb"] = cpool.tile([SCH, 4, TCH], U8,
                                                  name=f"mask{rep}",
                                                  tag="mask", bufs=1)
            ones_col = res["ones_col"] = cpool.tile([128, 1], BF16,
                                                    name=f"onec{rep}", tag="onec")
            ones_t = res["ones_t"] = cpool.tile([128, TCH], BF16,
                                                name=f"onet{rep}", tag="onet")
            V_st = res["V_st"] = vstp.tile([SCH, NSC, HL, HS + 1], BF16,
                                           name=f"V_st{rep}", tag="V")
            res["O_sb"] = osbp.tile([128, 2, NP, T], BF16,
                                    name=f"O_sb{rep}", tag="O")
            vsuf = res["vsuf"] = cpool.tile([1, 3, 2, HWID], BF16,
                                            name=f"vsuf{rep}", tag="vsuf")
            vpr = res["vpr"] = cpool.tile([1, NTC, 2, HWID], BF16,
                                          name=f"vpr{rep}", tag="vpr")

            ops = []
            for dc in range(NDC):
                def dma_in(dc=dc):
                    nc.sync.dma_start(wv_sb[:, dc, :],
                                      wv[dc * 128:(dc + 1) * 128, :])
                    nc.sync.dma_start(x_sb[:, dc, :],
                                      xT[dc * 128:(dc + 1) * 128, :])
                ops.append(dma_in)

            def dma_misc():
                nc.vector.memset(ones_col[:], 1.0)
                for dc in range(NDC):
                    nc.sync.dma_start(wq_sb[:, dc, :],
                                      wq[dc * 128:(dc + 1) * 128, :])
                for dc in range(NDC2):
                    nc.sync.dma_start(x8_sb[:, dc, :, :], xT8[:, dc, :, :])
                    nc.sync.dma_start(wk_sb[:, dc, :, :], wk[:, dc, :, :])
            ops.append(dma_misc)

            # V phase: dc-outer over groups of 4 s-chunks
            vstate = {}
            for grp in range(4):
                def v_alloc(grp=grp):
                    vstate[grp] = [
                        spool.tile([SCH, 2, HL, HS], F32, tag="ps",
                                   name=f"pv{rep}_{grp}_{i}") for i in range(2)]
                ops.append(v_alloc)
                for dc in range(NDC):
                    def v_mm(grp=grp, dc=dc):
                        pv = vstate[grp]
                        for i in range(2):
                            for u in range(2):
                                sc = 4 * grp + 2 * i + u
                                nc.tensor.matmul(
                                    pv[i][:, u, :, :],
                                    x_sb[:, dc, sc * 128:(sc + 1) * 128],
                                    wv_sb[:, dc, :],
                                    start=(dc == 0), stop=(dc == NDC - 1),
                                    skip_group_check=True)
                    ops.append(v_mm)
                if grp == 0:
                    def v_memset():
                        # first write of the rep into V_st (WARs on the
                        # previous rep's last attention reads)
                        nc.vector.memset(V_st[:, :, :, 0:1], 1.0)
                    ops.append(v_memset)
                def v_copy(grp=grp):
                    pv = vstate[grp]
                    for i in range(2):
                        for u in range(2):
                            nc.vector.tensor_copy(
                                V_st[:, 4 * grp + 2 * i + u, :, 1:HS + 1],
                                pv[i][:, u, :, :])
                ops.append(v_copy)

            # suffix sums over s-chunk tails (incl. masked counts in col 0)
            for tcb in range(3):
                for half in range(2):
                    def sfx(tcb=tcb, half=half):
                        psf = opool.tile([1, HWID], F32, tag="po",
                                         name=f"psf{rep}_{tcb}_{half}")
                        lo = 4 * (tcb + 1)
                        for c in range(lo, NSC):
                            nc.tensor.matmul(
                                psf[:], ones_col[:],
                                V_st[:, c, half * 4:(half + 1) * 4, :],
                                start=(c == lo), stop=(c == NSC - 1))
                        nc.vector.tensor_copy(vsuf[0:1, tcb, half, :], psf[:])
                    ops.append(sfx)
            # pair sums of chunks (4t+2, 4t+3): their t<256 region is all-ones
            for tcb in range(NTC):
                for half in range(2):
                    def prs(tcb=tcb, half=half):
                        ppr = opool.tile([1, HWID], F32, tag="po",
                                         name=f"ppr{rep}_{tcb}_{half}")
                        for ci, c in enumerate((4 * tcb + 2, 4 * tcb + 3)):
                            nc.tensor.matmul(
                                ppr[:], ones_col[:],
                                V_st[:, c, half * 4:(half + 1) * 4, :],
                                start=(ci == 0), stop=(ci == 1))
                        nc.vector.tensor_copy(vpr[0:1, tcb, half, :], ppr[:])
                    ops.append(prs)

            def dma_late():
                nc.vector.memset(ones_t[:], 1.0)
                for k in range(4):
                    nc.sync.dma_start(mask_sb[:, k, :], mask[k, :, :])
                for dc in range(NDC):
                    nc.sync.dma_start(wo_sb[:, dc, :],
                                      wo[dc * 128:(dc + 1) * 128, :])
            ops.append(dma_late)

            QT, KT, qops = qk_ops(res, rep, 0)
            res["qk0"] = (QT, KT)
            ops.extend(qops)
            return res, ops

        def norm_rest(res, rep, j, tcb, stg):
            # deferred tail of the normalize: off the po critical path
            def run():
                rp0, rbc, og = [], [], []
                for e in range(2):
                    rp0.append(sp.tile([1, TCH], F32, tag="rp0",
                                       name=f"rp0_{rep}_{j}_{tcb}_{e}"))
                    nc.vector.reciprocal(rp0[e][:], stg[e][0:1, :])
                for e in range(2):
                    rbc.append(sp.tile([HS + 1, TCH], F32, tag="rbc",
                                       name=f"rbc_{rep}_{j}_{tcb}_{e}"))
                    nc.gpsimd.partition_broadcast(
                        rbc[e][:], rp0[e][:], channels=HS + 1)
                for e in range(2):
                    og.append(sp.tile([HS + 1, TCH], BF16, tag="og",
                                      name=f"og_{rep}_{j}_{tcb}_{e}"))
                    nc.vector.tensor_tensor(
                        og[e][:], stg[e][:], rbc[e][:], MULT)
                for e in range(2):
                    nc.sync.dma_start(
                        res["o_my"][j][64 * e:64 * e + 64,
                                       tcb * TCH:(tcb + 1) * TCH],
                        og[e][1:HS + 1, :])
                if tcb == NTC - 1:
                    if collective:
                        nc.gpsimd.collective_compute(
                            "AllGather", BYPASS,
                            replica_groups=GROUPS,
                            ins=[res["o_my"][j][:]],
                            outs=[res["o_all"][j][:]],
                        )
                    for g in range(2):
                        src = (res["o_all"][j][g, :, :] if collective
                               else res["o_my"][j][:])
                        nc.sync.dma_start(res["O_sb"][:, g, j, :], src)
            return run

        def proj_ops(res, rep):
            """Output projection micro-ops (one per 128-row t-tile)."""
            O_sb, wo_sb = res["O_sb"], res["wo_sb"]
            jj_order = [g * 4 + jp for jp in range(NP) for g in range(2)]
            ops = []
            for tt in range(T // 128):
                def run(tt=tt):
                    pp = qpool.tile([128, TCH], F32,
                                    tag=("pq" if tt % 2 == 0 else "pk"),
                                    name=f"pp{rep}_{tt}")
                    for i, jj in enumerate(jj_order):
                        g, jp = jj // 4, jj % 4
                        nc.tensor.matmul(
                            pp[:],
                            O_sb[:, g, jp, tt * 128:(tt + 1) * 128],
                            wo_sb[:, jj, :],
                            start=(i == 0), stop=(i == NDC - 1))
                    ob = sp.tile([128, TCH], BF16, tag="ob",
                                 name=f"ob{rep}_{tt}")
                    nc.vector.tensor_copy(ob[:], pp[:])
                    nc.sync.dma_start(out[tt * 128:(tt + 1) * 128, :], ob[:])
                ops.append(run)
            return ops

        # ================= main pipeline =================
        res, pre = make_prelude(0)
        flush(pre)
        carry_proj = []

        for rep in range(reps):
            pending_norm = []
            for j in range(NP):
                if j + 1 < NP:
                    QTn, KTn, nxt = qk_ops(res, rep, j + 1)
                elif rep + 1 < reps:
                    res_next, nxt = make_prelude(rep + 1)
                    QTn = KTn = None
                else:
                    res_next, nxt, QTn, KTn = None, [], None, None
                QT, KT = res["qk0"] if j == 0 else (QT, KT)
                mask_sb = res["mask_sb"]
                ones_t = res["ones_t"]
                V_st = res["V_st"]
                vsuf, vpr = res["vsuf"], res["vpr"]
                # ---- attention for heads (2j, 2j+1) ----
                for tcb in range(NTC):
                    nv = 4 * (tcb + 1)
                    E = [ep.tile([SCH, 8, TCH], BF16, tag="E",
                                 name=f"E{rep}_{j}_{tcb}_{ee}")
                         for ee in range(2)]
                    po = [opool.tile([HS + 1, TCH], F32, tag="po",
                                     name=f"po{rep}_{j}_{tcb}_{ee}")
                          for ee in range(2)]
                    for cp in range(nv // 2):
                        ps = [None, None]
                        for e in range(2):
                            ps[e] = spool.tile(
                                [SCH, 2, TCH], F32, tag="ps",
                                name=f"ps{rep}_{j}_{tcb}_{cp}_{e}")
                        if cp == 1 and pending_norm:
                            pending_norm.pop(0)()
                        # coarse diagonal split: the second diagonal cp-group
                        # computes only t >= 256; its t < 256 all-ones region
                        # is added as one rank-1 below
                        g = cp - 2 * tcb
                        t0 = 256 if g == 1 else 0
                        for u in range(2):
                            for e in range(2):
                                c = 2 * cp + u
                                nc.tensor.matmul(
                                    ps[e][:, u, t0:TCH],
                                    KT[64 * e:64 * e + 64, c // 4,
                                       (c % 4) * SCH:(c % 4 + 1) * SCH],
                                    QT[64 * e:64 * e + 64, tcb, t0:TCH],
                                    start=True, stop=True)
                        flush(nxt, 2)
                        if not nxt and carry_proj:
                            # only once the QK/prelude stream for this window
                            # has fully drained: proj shares the qpool tags,
                            # and interleaving the allocations deadlocks the
                            # in-order queues
                            flush(carry_proj, 2)
                        for e in range(2):
                            cm = (2 * cp) % 8
                            nc.scalar.activation(
                                E[e][:, cm:cm + 2, t0:TCH],
                                ps[e][:, :, t0:TCH], EXP, scale=1.0 / 32.0)
                            if g >= 0:
                                k0, k1 = 2 * g, 2 * g + 1
                                nc.vector.copy_predicated(
                                    E[e][:, cm, t0:t0 + 128],
                                    mask_sb[:, k0, t0:t0 + 128],
                                    ones_t[:, 0:128])
                                nc.vector.copy_predicated(
                                    E[e][:, cm + 1, t0:t0 + 256],
                                    mask_sb[:, k1, t0:t0 + 256],
                                    ones_t[:, 0:256])
                        for e in range(2):
                            h = 2 * j + e
                            for u in range(2):
                                c = 2 * cp + u
                                nc.tensor.matmul(
                                    po[e][:, t0:TCH],
                                    V_st[:, c, h, :],
                                    E[e][:, c % 8, t0:TCH],
                                    start=(c == 0), stop=False,
                                    skip_group_check=True)
                            if g == 1:
                                # all-ones region of chunks (4t+2, 4t+3)
                                nc.tensor.matmul(
                                    po[e][:, 0:256],
                                    vpr[0:1, tcb, j // 2,
                                        (h % 4) * (HS + 1):
                                        (h % 4 + 1) * (HS + 1)],
                                    ones_t[0:1, 0:256],
                                    start=False, stop=(tcb == 3),
                                    skip_group_check=True)
                        flush(nxt, 2)
                    if tcb < 3:
                        for e in range(2):
                            h = 2 * j + e
                            nc.tensor.matmul(
                                po[e][:],
                                vsuf[0:1, tcb, j // 2,
                                     (h % 4) * (HS + 1):
                                     (h % 4 + 1) * (HS + 1)],
                                ones_t[0:1, :],
                                start=False, stop=True,
                                skip_group_check=True)
                    # normalize: fast psum->sbuf staging frees the po bank;
                    # the recip/broadcast/mult tail is deferred
                    stg = []
                    for e in range(2):
                        stg.append(sp.tile([HS + 1, TCH], F32, tag="stg",
                                           name=f"stg{rep}_{j}_{tcb}_{e}"))
                        nc.vector.tensor_copy(stg[e][:], po[e][:])
                    pending_norm.append(norm_rest(res, rep, j, tcb, stg))
                while pending_norm:
                    pending_norm.pop(0)()
                flush(nxt)
                if j + 1 < NP:
                    QT, KT = QTn, KTn
            # this rep's projection: interleave into the next rep's first
            # attention pair (or flush now on the last rep)
            carry_proj.extend(proj_ops(res, rep))
            if rep + 1 < reps:
                res = res_next
            else:
                flush(carry_proj)

    nc.compile()
    return nc


def make_mask():
    p = np.arange(SCH)[:, None]
    f = np.arange(TCH)[None, :]
    return np.stack([(p + 128 * k > f) for k in range(4)]).astype(np.uint8)


def make_in_maps(x, W_qkv, W_out):
    x = np.asarray(x, dtype=np.float32)
    W_qkv = np.asarray(W_qkv, dtype=np.float32)
    W_out = np.asarray(W_out, dtype=np.float32)
    mask = make_mask()
    in_maps = []
    for c in range(8):
        b, hg = c // 2, c % 2
        heads = slice(hg * HL, (hg + 1) * HL)
        wq_h = W_qkv[heads, :, 0:HS].transpose(1, 0, 2).reshape(D, HL * HS)
        wk_h = W_qkv[heads, :, HS:2 * HS].transpose(1, 0, 2).reshape(D, HL * HS)
        wv_h = W_qkv[heads, :, 2 * HS:3 * HS].transpose(1, 0, 2).reshape(D, HL * HS)
        bf = ml_dtypes.bfloat16
        f8 = ml_dtypes.float8_e4m3

        def pack(a):
            # [D, n] -> [128, 4, 2, n] with d = dcp*256 + 2*p + i
            return np.ascontiguousarray(
                a.reshape(4, 128, 2, a.shape[1]).transpose(1, 0, 2, 3)
            ).astype(f8)

        in_maps.append({
            "xT": np.ascontiguousarray(x[b].T).astype(bf),
            "xT8": pack(x[b].T),
            "wq": np.ascontiguousarray(wq_h).astype(bf),
            "wk": pack(wk_h),
            "wv": np.ascontiguousarray(wv_h).astype(bf),
            "wo": np.ascontiguousarray(
                W_out[:, hg * TCH:(hg + 1) * TCH]).astype(bf),
            "mask": mask,
        })
    return in_maps


_NC_CACHE = {}


def get_nc():
    if "nc" not in _NC_CACHE:
        _NC_CACHE["nc"] = build()
    return _NC_CACHE["nc"]


def kernel(x, W_qkv, W_out):
    nc = get_nc()
    in_maps = make_in_maps(x, W_qkv, W_out)
    res = run_bass_kernel_spmd(nc, in_maps, list(range(8)))
    out = np.empty((B, T, D), dtype=np.float32)
    for b in range(B):
        out[b, :, 0:TCH] = np.asarray(res.results[2 * b]["out"],
                                      dtype=np.float32)
        out[b, :, TCH:D] = np.asarray(res.results[2 * b + 1]["out"],
                                      dtype=np.float32)
    return out
